# revision 17
# baseline (speedup 1.0000x reference)
"""Trainium2 Bass kernel: multi-head attention block (DiyTransformer).

Full-input contract: kernel(**inputs) takes the unsharded inputs and returns
the full [2, 2048, 1024] output. Internally shards 16 heads across 8
NeuronCores (2 heads = one 128-wide feature slice per core).

Math (reference):
  q = x @ wq.T + bq ; k = x @ wk.T + bk ; v = x @ wv.T + bv   (per-head split)
  out_h = softmax(q_h k_h^T / 8) v_h ;  y = concat(out_h) @ wo.T + bo

Simplifications:
  - k bias cancels in softmax; dropped. v bias folded into host-side
    bo_eff = bo + bv @ wo.T (softmax rows sum to 1). 1/8 scale folded into
    wq/bq on the host.
  - scores are computed transposed (scoresT[k_pos, q] = k @ qT) so no PE
    transposes are needed; a ones-column appended per head slot in v makes
    the PV matmul emit the softmax denominator for free (psum row 64).

v2 design (three-engine balance; ScalarE exp is the floor otherwise):
  - 5 of 16 score chunks per q-block are exponentiated on the Vector engine
    with a Schraudolph-style bit trick instead of ScalarE:
      bf16_bits(e^x) ~= int16(x * 128*log2(e) + 16256)
    written via an int16-bitcast view of the bf16 exp buffer. This offloads
    ~30% of the exp work from the saturated ScalarE (~1.15us/chunk) to DVE
    slack, at ~8.4e-3 end-to-end rel err (vs 1.9e-3 all-ScalarE).
  - PSUM (8 banks): A[128,1024] + B[128,1024] alternate for ScalarE exp
    double-buffering; C[128,512]x2 rotates DVE-exp chunks plus all "rider"
    matmuls (q/k/v projections, output projection); PV accumulators
    [128,512]x2 (one per head, live per block).
  - v is projected as vT stripes (N=512 matmuls, 4x fewer instructions than
    natural-layout N=128) and transposed into PV layout with XBAR
    DMA-transposes on the scalar-engine DMA ring.
  - the whole kernel is software-pipelined: projections for batch n+1 and
    the output projection of block i-1 ride the C-pool rotation inside
    block i, so PE/ACT/DVE all stay busy.
"""

import sys

sys.path.insert(0, "/opt/trn_rl_repo")

import zlib
from collections import deque

import numpy as np
import ml_dtypes

# The axon terminal caches compiled executables by module name + I/O
# signature only, so a changed kernel with unchanged tensor shapes silently
# reuses the stale executable. Bust it by adding a dummy input whose shape
# encodes a hash of this file's source.
with open(__file__, "rb") as _f:
    _VTAG = (zlib.crc32(_f.read()) % 4093) + 3

D = 1024          # embed dim
NH = 16           # total heads
DH = 64           # head dim
NB = 2            # batch
S = 2048          # seq len
M = NB * S        # 4096 flattened rows
N_CORES = 8
HPC = 2           # heads per core
FS = HPC * DH     # 128 per-core feature slice
DCH = D // 128    # 8 contraction chunks
SCALE = 1.0 / np.sqrt(DH)
NCH = S // 128    # 16 k chunks per batch
VSLOT = 2 * 65    # vv bytes per chunk slot: [h0 v64|one][h1 v64|one]

BF16 = ml_dtypes.bfloat16

_compiled = None

# score chunks exponentiated on DVE via the bit trick (rest on ScalarE)
DVE_CHUNKS = (1, 4, 7, 10, 13, 15)
TRICK_MUL = float(128.0 / np.log(2.0))
TRICK_ADD = 16256.0


def _build(repeat=1, dve_chunks=DVE_CHUNKS, use_vtrans=False, fancy_dma=True, vtag_bump=0):
    import concourse.bass as bass
    import concourse.tile as tile
    from concourse import bacc, mybir

    f32 = mybir.dt.float32
    bf16 = mybir.dt.bfloat16
    i16 = mybir.dt.int16
    Exp = mybir.ActivationFunctionType.Exp
    ActCopy = mybir.ActivationFunctionType.Copy
    ActIdent = mybir.ActivationFunctionType.Identity
    MUL = mybir.AluOpType.mult
    ADD = mybir.AluOpType.add

    nc = bacc.Bacc("TRN2", target_bir_lowering=False, debug=False,
                   num_devices=N_CORES)

    xT_d = nc.dram_tensor("xT", [D, M], bf16, kind="ExternalInput").ap()
    wq_d = nc.dram_tensor("wqT", [D, FS], bf16, kind="ExternalInput").ap()
    wk_d = nc.dram_tensor("wkT", [D, FS], bf16, kind="ExternalInput").ap()
    wv_d = nc.dram_tensor("wvT", [D, FS], bf16, kind="ExternalInput").ap()
    wo_d = nc.dram_tensor("woT", [FS, D], bf16, kind="ExternalInput").ap()
    bq_d = nc.dram_tensor("bq", [FS, 1], f32, kind="ExternalInput").ap()
    nc.dram_tensor("vtag", [1, _VTAG + (repeat - 1) * 4096 + vtag_bump], f32,
                   kind="ExternalInput")
    out_d = nc.dram_tensor("out", [M, D], f32, kind="ExternalOutput").ap()

    dve_set = set(dve_chunks)
    act_chunks = [c for c in range(NCH) if c not in dve_set]
    pool_of = {c: i % 2 for i, c in enumerate(act_chunks)}  # 0=A, 1=B

    with tile.TileContext(nc) as tc:
        with (
            tc.tile_pool(name="persist", bufs=1) as persist,
            tc.tile_pool(name="et", bufs=1) as et_pool,
            tc.tile_pool(name="vt", bufs=2) as vt_pool,
            tc.tile_pool(name="oc", bufs=3) as oc_pool,
            tc.tile_pool(name="oT", bufs=2) as oT_pool,
            tc.tile_pool(name="smalls", bufs=2) as smalls,
            tc.tile_pool(name="ps_a", bufs=1, space="PSUM") as ps_a,
            tc.tile_pool(name="ps_b", bufs=1, space="PSUM") as ps_b,
            tc.tile_pool(name="ps_c", bufs=1, space="PSUM") as ps_c,
            tc.tile_pool(name="ps_pv", bufs=2, space="PSUM") as ps_pv,
        ):
            for _rep in range(repeat):
                # ---- persistent SBUF tiles ----
                xT = persist.tile([128, DCH * M], bf16, tag="xT")
                wq = persist.tile([128, D], bf16, tag="wq")
                wk = persist.tile([128, D], bf16, tag="wk")
                wv = persist.tile([128, D], bf16, tag="wv")
                wo = persist.tile([128, D], bf16, tag="wo")
                bq = persist.tile([FS, 1], f32, tag="bq")
                qT = persist.tile([128, M], bf16, tag="qT")
                kT = persist.tile([128, M], bf16, tag="kT")
                vv = persist.tile([128, NB * NCH * VSLOT], bf16, tag="vv")

                # ---- input DMAs: weights first, then xT stripe-ordered ----
                def wload(dst, src_d):
                    da = dst[:, 0:128]
                    sa = src_d[0:128, 0:128]
                    nc.sync.dma_start(
                        bass.AP(da.tensor, da.offset,
                                [[da.ap[0][0], 128], [128, DCH], [1, 128]]),
                        bass.AP(sa.tensor, sa.offset,
                                [[FS, 128], [128 * FS, DCH], [1, 128]]))

                if fancy_dma:
                    pass
                else:
                    for d in range(DCH):
                        sl = slice(d * 128, (d + 1) * 128)
                        nc.sync.dma_start(wk[:, sl], wk_d[sl, :])
                        nc.sync.dma_start(wv[:, sl], wv_d[sl, :])
                        nc.sync.dma_start(wq[:, sl], wq_d[sl, :])
                if not fancy_dma:
                    nc.sync.dma_start(wo[:, :], wo_d[:, :])
                    nc.sync.dma_start(bq[:, :], bq_d[:, :])
                nc.vector.memset(vv[:, :], 1.0)

                # xT stripes: per (batch, 512-col stripe), 2 DMAs of 4
                # d-chunks each, in the order the projection riders consume.
                def xload(n, jj):
                    base = n * S + jj * 512
                    for dh in range(2):
                        d0 = dh * 4
                        da = xT[:, d0 * M + base: d0 * M + base + 512]
                        sa = xT_d[d0 * 128:(d0 + 1) * 128, base:base + 512]
                        nc.sync.dma_start(
                            bass.AP(da.tensor, da.offset,
                                    [[da.ap[0][0], 128], [M, 4], [1, 512]]),
                            bass.AP(sa.tensor, sa.offset,
                                    [[M, 128], [128 * M, 4], [1, 512]]))

                if fancy_dma:
                    wload(wk, wk_d)
                    nc.sync.dma_start(bq[:, :], bq_d[:, :])
                    xload(0, 0)
                    wload(wq, wq_d)
                    xload(0, 1)
                    wload(wv, wv_d)
                    nc.sync.dma_start(wo[:, :], wo_d[:, :])
                    xload(0, 2)
                    xload(0, 3)
                    for jj in range(4):
                        xload(1, jj)
                else:
                    for d in range(DCH):
                        nc.sync.dma_start(xT[:, d * M:(d + 1) * M],
                                          xT_d[d * 128:(d + 1) * 128, :])

                # ---- rider emitters (all use the C psum rotation) ----
                def kstripe(n, jj):
                    def go():
                        pk = ps_c.tile([128, 512], f32, tag="c")
                        base = n * S + jj * 512
                        for d in range(DCH):
                            nc.tensor.matmul(pk[:, :], wk[:, d * 128:(d + 1) * 128],
                                             xT[:, d * M + base: d * M + base + 512],
                                             start=(d == 0), stop=(d == DCH - 1))
                        nc.scalar.activation(kT[:, base:base + 512],
                                             pk[:, :], ActCopy)
                    return go

                def qstripe(n, jj):
                    def go():
                        pq = ps_c.tile([128, 512], f32, tag="c")
                        base = n * S + jj * 512
                        for d in range(DCH):
                            nc.tensor.matmul(pq[:, :], wq[:, d * 128:(d + 1) * 128],
                                             xT[:, d * M + base: d * M + base + 512],
                                             start=(d == 0), stop=(d == DCH - 1))
                        nc.scalar.activation(qT[:, base:base + 512],
                                             pq[:, :], ActIdent,
                                             bias=bq[:, 0:1])
                    return go

                def vstripe(n, jj):
                    if not use_vtrans:
                        def go_direct():
                            for cpair in range(2):
                                pv_ = ps_c.tile([128, 512], f32, tag="c")
                                for cc in range(2):
                                    c = jj * 4 + cpair * 2 + cc
                                    base = n * S + c * 128
                                    tgt = pv_[:, cc * 128:(cc + 1) * 128]
                                    for d in range(DCH):
                                        nc.tensor.matmul(
                                            tgt, xT[:, d * M + base: d * M + base + 128],
                                            wv[:, d * 128:(d + 1) * 128],
                                            start=(d == 0), stop=(d == DCH - 1))
                                c0 = jj * 4 + cpair * 2
                                vs0 = (n * NCH + c0) * VSLOT
                                dv = vv[:, vs0:vs0 + 64]
                                sv = pv_[:, 0:64]
                                nc.vector.tensor_copy(
                                    bass.AP(dv.tensor, dv.offset,
                                            [[dv.ap[0][0], 128], [VSLOT, 2],
                                             [65, 2], [1, 64]]),
                                    bass.AP(sv.tensor, sv.offset,
                                            [[sv.ap[0][0], 128], [128, 2],
                                             [64, 2], [1, 64]]))
                        return go_direct
                    def go():
                        pv_ = ps_c.tile([128, 512], f32, tag="c")
                        base = n * S + jj * 512
                        for d in range(DCH):
                            nc.tensor.matmul(pv_[:, :], wv[:, d * 128:(d + 1) * 128],
                                             xT[:, d * M + base: d * M + base + 512],
                                             start=(d == 0), stop=(d == DCH - 1))
                        vts = vt_pool.tile([128, 512], bf16, tag="vt")
                        nc.vector.tensor_copy(vts[:, :], pv_[:, :])
                        for cc in range(4):
                            c = jj * 4 + cc
                            for h in range(HPC):
                                vs = (n * NCH + c) * VSLOT + h * 65
                                nc.sync.dma_start_transpose(
                                    vv[:, vs:vs + 64],
                                    vts[h * 64:(h + 1) * 64, cc * 128:(cc + 1) * 128])
                    return go

                def po_rider(oTt, q0, t):
                    def go():
                        lo = ps_c.tile([128, 512], f32, tag="c")
                        nc.tensor.matmul(lo[:, :], oTt[:, t * 128:(t + 1) * 128],
                                         wo[:, 0:512], start=True, stop=True)
                        hi = ps_c.tile([128, 512], f32, tag="c")
                        nc.tensor.matmul(hi[:, :], oTt[:, t * 128:(t + 1) * 128],
                                         wo[:, 512:1024], start=True, stop=True)
                        oc = oc_pool.tile([128, 1024], f32, tag="oc")
                        if t < 1:
                            nc.scalar.activation(oc[:, 0:512], lo[:, :], ActCopy)
                            nc.scalar.activation(oc[:, 512:1024], hi[:, :], ActCopy)
                        else:
                            nc.vector.tensor_copy(oc[:, 0:512], lo[:, :])
                            nc.vector.tensor_copy(oc[:, 512:1024], hi[:, :])
                        nc.sync.dma_start(
                            out_d[q0 + t * 128: q0 + (t + 1) * 128, :], oc[:, :])
                    return go

                riders = deque()

                def rider_point():
                    if riders:
                        riders.popleft()()

                # ---- attention block ----
                def emit_block(n, j):
                    q0 = n * S + j * 512
                    et = et_pool.tile([128, NCH * HPC * 512], bf16, tag="et")
                    pvs = [ps_pv.tile([128, 512], f32, tag="pv", name=f"pv{h}")
                           for h in range(HPC)]
                    oTt = oT_pool.tile([128, 512], bf16, tag="oT")

                    def emit_pv(c):
                        for h in range(HPC):
                            vs = (n * NCH + c) * VSLOT + h * 65
                            nc.tensor.matmul(
                                pvs[h][0:65, :], vv[:, vs:vs + 65],
                                et[:, (c * HPC + h) * 512:(c * HPC + h + 1) * 512],
                                start=(c == 0), stop=(c == NCH - 1))

                    pv_cursor = 0
                    for c in range(NCH):
                        k0 = n * S + c * 128
                        if c in dve_set:
                            ct = ps_c.tile([128, 1024], f32, tag="c")
                            for h in range(HPC):
                                hp = slice(h * DH, (h + 1) * DH)
                                nc.tensor.matmul(ct[:, h * 512:(h + 1) * 512],
                                                 kT[hp, k0:k0 + 128],
                                                 qT[hp, q0:q0 + 512],
                                                 start=True, stop=True)
                            eslot = et[:, c * HPC * 512:(c + 1) * HPC * 512]
                            nc.vector.tensor_scalar(
                                eslot.bitcast(i16), ct[:, :],
                                TRICK_MUL, TRICK_ADD, MUL, ADD)
                            rider_point()
                        else:
                            pool = ps_a if pool_of[c] == 0 else ps_b
                            ps = pool.tile([128, HPC * 512], f32,
                                           tag="a" if pool_of[c] == 0 else "b")
                            for h in range(HPC):
                                hp = slice(h * DH, (h + 1) * DH)
                                nc.tensor.matmul(ps[:, h * 512:(h + 1) * 512],
                                                 kT[hp, k0:k0 + 128],
                                                 qT[hp, q0:q0 + 512],
                                                 start=True, stop=True)
                            nc.scalar.activation(
                                et[:, c * HPC * 512:(c + 1) * HPC * 512],
                                ps[:, :], Exp)
                            if c in (3, 6, 9, 12):
                                rider_point()
                        while pv_cursor <= c - 2:
                            emit_pv(pv_cursor)
                            pv_cursor += 1
                    while pv_cursor < NCH:
                        emit_pv(pv_cursor)
                        pv_cursor += 1

                    # denominators -> reciprocal -> broadcast -> normalize
                    den = smalls.tile([1, 1024], f32, tag="den")
                    for h in range(HPC):
                        nc.vector.tensor_copy(den[:, h * 512:(h + 1) * 512],
                                              pvs[h][64:65, :])
                    recip = smalls.tile([1, 1024], f32, tag="recip")
                    nc.vector.reciprocal_approx_fast(recip[:, :], den[:, :])
                    bc = smalls.tile([64, 1024], f32, tag="bc")
                    rap = recip[:, :]
                    nc.sync.dma_start(
                        bc[:, :],
                        bass.AP(rap.tensor, rap.offset,
                                [[rap.ap[0][0], 1], [0, 64], [1, 1024]]))
                    for h in range(HPC):
                        nc.vector.tensor_mul(oTt[h * DH:(h + 1) * DH, :],
                                             pvs[h][0:64, :],
                                             bc[:, h * 512:(h + 1) * 512])
                    return oTt, q0

                # ---- software pipeline over the 8 blocks ----
                # prologue: first k/q/v stripes for batch 0
                kstripe(0, 0)()
                qstripe(0, 0)()
                vstripe(0, 0)()

                # per-block rider schedules (projections for upcoming blocks)
                sched = [
                    [kstripe(0, 1), vstripe(0, 1), kstripe(0, 2), vstripe(0, 2),
                     kstripe(0, 3), vstripe(0, 3), qstripe(0, 1)],
                    [kstripe(1, 0), vstripe(1, 0), qstripe(0, 2)],
                    [kstripe(1, 1), vstripe(1, 1), qstripe(0, 3)],
                    [kstripe(1, 2), vstripe(1, 2), kstripe(1, 3), qstripe(1, 0)],
                    [vstripe(1, 3), qstripe(1, 1)],
                    [qstripe(1, 2)],
                    [qstripe(1, 3)],
                    [],
                ]

                prev_po = []
                for bi in range(NB * 4):
                    n, j = divmod(bi, 4)
                    proj = sched[bi]
                    merged = []
                    pi = 0
                    for r in prev_po:
                        merged.append(r)
                        if pi < len(proj):
                            merged.append(proj[pi])
                            pi += 1
                    merged.extend(proj[pi:])
                    riders.extend(merged)
                    oTt, q0 = emit_block(n, j)
                    while riders:  # flush stragglers at block boundary
                        riders.popleft()()
                    prev_po = [po_rider(oTt, q0, t) for t in range(4)]
                for r in prev_po:
                    r()

    nc.compile()
    return nc


def _get_compiled():
    global _compiled
    if _compiled is None:
        _compiled = _build()
    return _compiled


def _prep_in_maps(x, wq, bq, wk, wv, wo):
    xT = np.ascontiguousarray(x.reshape(M, D).T).astype(BF16)
    maps = []
    for i in range(N_CORES):
        rs = slice(i * FS, (i + 1) * FS)
        maps.append({
            "xT": xT,
            "wqT": np.ascontiguousarray((wq[rs, :] * SCALE).T).astype(BF16),
            "wkT": np.ascontiguousarray(wk[rs, :].T).astype(BF16),
            "wvT": np.ascontiguousarray(wv[rs, :].T).astype(BF16),
            "woT": np.ascontiguousarray(wo[:, rs].T).astype(BF16),
            "bq": (bq[rs] * SCALE).astype(np.float32).reshape(FS, 1),
            "vtag": np.zeros((1, _VTAG), np.float32),
        })
    return maps


def kernel(x, wq, bq, wk, bk, wv, bv, wo, bo, _want_results=False, _trace=False):
    from concourse.bass_utils import run_bass_kernel_spmd

    x = np.asarray(x, dtype=np.float32)
    wq = np.asarray(wq, dtype=np.float32)
    bq = np.asarray(bq, dtype=np.float32)
    wk = np.asarray(wk, dtype=np.float32)
    wv = np.asarray(wv, dtype=np.float32)
    wo = np.asarray(wo, dtype=np.float32)
    bv = np.asarray(bv, dtype=np.float32)
    bo = np.asarray(bo, dtype=np.float32)

    nc = _get_compiled()
    in_maps = _prep_in_maps(x, wq, bq, wk, wv, wo)
    res = None
    for attempt in range(3):
        try:
            res = run_bass_kernel_spmd(nc, in_maps, list(range(N_CORES)),
                                       trace=_trace)
            break
        except Exception:
            # the shared device occasionally reports
            # NRT_EXEC_UNIT_UNRECOVERABLE transiently; back off and retry
            if attempt == 2:
                raise
            import time as _time
            _time.sleep(15)

    acc = np.zeros((M, D), dtype=np.float32)
    for i in range(N_CORES):
        acc += res.results[i]["out"]
    acc += bo + bv @ wo.T
    out = acc.reshape(NB, S, D)
    if _want_results:
        return out, res
    return out
